# revision 15
# baseline (speedup 1.0000x reference)
"""BitLinear (RMSNorm + int8 act quant + ternary weight quant + GEMM) on 8 TRN2 cores.

Sharding: 2 token-groups x 4 dout-groups. Each core:
  - x shard [4096, 2048] (token-parallel)
  - wT shard [2048, 2048] = weight[og*2048:(og+1)*2048, :].T  (host pre-transposed layout)
  - wsc shard [1024, 2048] = weight[c*1024:(c+1)*1024, :]     (for global mean|w| AllReduce)
  - norm_weight replicated across 128 partitions
Device pipeline per core:
  pass A: sum|wsc| -> AllReduce(8 cores) -> w_scale, inv_w_scale
  pass B: quantize wT to ternary bf16 (magic-constant RNE round + clip)
  x loop: per 128-token tile: sum(x^2) (ACT Square+accum), max|x*g| (DVE reduce),
          per-token scalars, q = round((x*g)*m) via magic round -> bf16,
          DMA-transpose q to [d, t] layout, 64 bf16 matmuls into PSUM,
          scale by x_scale*w_scale on PSUM->SBUF copy, DMA out.
The quantized GEMM is exact: x_q in [-127,127] and w_q in {-1,0,1} are exactly
representable in bf16 and PSUM accumulates in f32 (|sums| < 2^24).
"""

import sys

if "/opt/trn_rl_repo" not in sys.path:
    sys.path.insert(0, "/opt/trn_rl_repo")

import numpy as np

# ---------------------------------------------------------------- config

N_CORES = 8
TG, OG = 2, 4            # token groups x dout groups
B, S, DIN, DOUT = 4, 2048, 2048, 8192
TOKENS = B * S           # 8192
T_SH = TOKENS // TG      # 4096 tokens per core
O_SH = DOUT // OG        # 2048 dout per core
WSC_ROWS = DOUT // N_CORES  # 1024 rows of w per core for the scale pass

P = 128                  # partitions
EPS_NORM = 1e-6
EPS_SCALE = 1e-8
QB = 127.0
C_MAGIC = 12582912.0     # 1.5 * 2^23 : float32 RNE integer-rounding constant
N_W = float(DOUT * DIN)  # elements of weight for the global mean


def build_bass(t_sh=T_SH, din=DIN, o_sh=O_SH, wsc_rows=WSC_ROWS, n_w=N_W,
               n_cores=N_CORES, group=8):
    """Build the per-core SPMD Bass graph. Shapes parametrized for sim tests."""
    import concourse.bass as bass
    import concourse.bacc as bacc
    import concourse.mybir as mybir
    from concourse import tile

    fp32 = mybir.dt.float32
    bf16 = mybir.dt.bfloat16
    Alu = mybir.AluOpType
    Act = mybir.ActivationFunctionType

    t_tiles = t_sh // P          # token tiles
    k_tiles = din // P           # contraction tiles
    oc_sz = 512 if o_sh >= 512 else o_sh
    oc_chunks = o_sh // oc_sz    # PSUM output chunks per token tile
    wsc_tiles = wsc_rows // P

    nc = bacc.Bacc("TRN2", target_bir_lowering=False, debug=False,
                   num_devices=n_cores)

    x_d = nc.dram_tensor("x", [t_sh, din], fp32, kind="ExternalInput")
    wt_d = nc.dram_tensor("wt", [din, o_sh], fp32, kind="ExternalInput")
    wsc_d = nc.dram_tensor("wsc", [wsc_rows, din], fp32, kind="ExternalInput")
    gw_d = nc.dram_tensor("gw", [P, din], fp32, kind="ExternalInput")
    out_d = nc.dram_tensor("out", [t_sh, o_sh], fp32, kind="ExternalOutput")

    # collective bounce buffers (internal DRAM)
    pin_d = nc.dram_tensor("cc_in", [P, 1], fp32)
    pout_d = nc.dram_tensor("cc_out", [P, 1], fp32)
    warm_in_d = nc.dram_tensor("cc_warm_in", [P, 1], fp32)
    warm_out_d = nc.dram_tensor("cc_warm_out", [P, 1], fp32)

    with tile.TileContext(nc) as tc:
        with (
            tc.tile_pool(name="persist", bufs=1) as persist,
            tc.tile_pool(name="xin", bufs=2) as xin_pool,
            tc.tile_pool(name="ybuf", bufs=2) as y_pool,
            tc.tile_pool(name="t1buf", bufs=2) as t1_pool,
            tc.tile_pool(name="qbuf", bufs=2) as q_pool,
            tc.tile_pool(name="qtbuf", bufs=3) as qt_pool,
            tc.tile_pool(name="obuf", bufs=2) as out_pool,
            tc.tile_pool(name="wtq", bufs=16) as wtq_pool,
            tc.tile_pool(name="small", bufs=4) as small,
            tc.tile_pool(name="psum", bufs=4, space="PSUM") as psum_pool,
        ):
            # Warm-up collective issued first: the ncfw path pays a one-time
            # ~60-70us arming barrier on the first collective of a NEFF. Absorb
            # it on dummy buffers while pass A's DMAs run, so the real
            # AllReduce below only pays the ~20us op cost.
            zsb = small.tile([P, 1], fp32, name="zsb")
            nc.gpsimd.memset(zsb[:], 0.0)
            nc.scalar.dma_start(warm_in_d[:], zsb[:])
            nc.gpsimd.collective_compute(
                "AllReduce", Alu.add,
                replica_groups=[list(range(n_cores))],
                ins=[warm_in_d[:]], outs=[warm_out_d[:]],
            )

            # ---------------- persistent tiles
            gw_sb = persist.tile([P, din], fp32)
            nc.scalar.dma_start(gw_sb[:], gw_d[:])
            ones_sb = persist.tile([P, P], fp32)
            nc.gpsimd.memset(ones_sb[:], 1.0)
            # per-k quantized transposed weight blocks [d_lo, o]
            wq = [persist.tile([P, o_sh], bf16, name=f"wq{k}") for k in range(k_tiles)]
            # per-token stats, one column per token tile
            sumsq_t = persist.tile([P, t_tiles], fp32)
            amax_t = persist.tile([P, t_tiles], fp32)
            m_t = persist.tile([P, t_tiles], fp32)
            alpha_t = persist.tile([P, t_tiles], fp32)

            # ---------------- pass A: global sum |w|
            wacc = persist.tile([P, wsc_tiles], fp32)
            for j in range(wsc_tiles):
                wtile = out_pool.tile([P, din], fp32, tag="o")
                nc.scalar.dma_start(wtile[:], wsc_d[j * P:(j + 1) * P, :])
                scr = t1_pool.tile([P, din], fp32, tag="t1")
                nc.scalar.activation(scr[:], wtile[:], Act.Abs,
                                     accum_out=wacc[:, j:j + 1])
            wpart = small.tile([P, 1], fp32)
            nc.vector.tensor_reduce(out=wpart[:], in_=wacc[:], op=Alu.add,
                                    axis=mybir.AxisListType.X)
            nc.scalar.dma_start(pin_d[:], wpart[:])
            nc.gpsimd.collective_compute(
                "AllReduce", Alu.add,
                replica_groups=[list(range(n_cores))],
                ins=[pin_d[:]], outs=[pout_d[:]],
            )
            wsum_all = small.tile([P, 1], fp32)
            nc.scalar.dma_start(wsum_all[:], pout_d[:])
            # cross-partition sum + broadcast via ones matmul
            psum_s = psum_pool.tile([P, 512], fp32, tag="ps", name="psum_s")
            nc.tensor.matmul(psum_s[:, 0:1], ones_sb[:], wsum_all[:],
                             start=True, stop=True)
            ssum = small.tile([P, 1], fp32)
            nc.vector.tensor_copy(ssum[:], psum_s[:, 0:1])
            ws = small.tile([P, 1], fp32)   # w_scale per partition (all equal)
            nc.vector.tensor_scalar(out=ws[:], in0=ssum[:], scalar1=1.0 / n_w,
                                    scalar2=EPS_SCALE, op0=Alu.mult, op1=Alu.add)
            inv_ws = small.tile([P, 1], fp32)
            nc.vector.reciprocal(inv_ws[:], ws[:])

            # ---------------- pass B: quantize wT -> ternary bf16.
            # Quarter tiles ([P, oc_sz]) through a dedicated 16-slot pool so
            # loads prefetch during the AllReduce; oc-major order so the
            # first oc chunk completes first and matmuls start early.
            for oc in range(oc_chunks):
                osl = slice(oc * oc_sz, (oc + 1) * oc_sz)
                for k in range(k_tiles):
                    wtile = wtq_pool.tile([P, oc_sz], fp32, tag="wtq")
                    nc.scalar.dma_start(wtile[:], wt_d[k * P:(k + 1) * P, osl])
                    tw1 = wtq_pool.tile([P, oc_sz], fp32, tag="tw1", bufs=2)
                    nc.vector.tensor_scalar(out=tw1[:], in0=wtile[:],
                                            scalar1=inv_ws[:], scalar2=C_MAGIC,
                                            op0=Alu.mult, op1=Alu.add)
                    tw2 = wtq_pool.tile([P, oc_sz], fp32, tag="tw2", bufs=2)
                    nc.vector.tensor_scalar(out=tw2[:], in0=tw1[:],
                                            scalar1=C_MAGIC, scalar2=1.0,
                                            op0=Alu.subtract, op1=Alu.min)
                    nc.vector.tensor_scalar(out=wq[k][:, osl], in0=tw2[:],
                                            scalar1=-1.0, scalar2=None,
                                            op0=Alu.max)

            # ---------------- x loop (fully per-tile; avoids pool-slot cycles)
            for i in range(t_tiles):
                    xt = xin_pool.tile([P, din], fp32, tag="xin")
                    nc.scalar.dma_start(xt[:], x_d[i * P:(i + 1) * P, :])
                    yt = y_pool.tile([P, din], fp32, tag="y")
                    nc.vector.tensor_tensor(out=yt[:], in0=xt[:], in1=gw_sb[:],
                                            op=Alu.mult)
                    scr = t1_pool.tile([P, din], fp32, tag="t1")
                    nc.scalar.activation(scr[:], xt[:], Act.Square,
                                         accum_out=sumsq_t[:, i:i + 1])
                    nc.vector.tensor_reduce(out=amax_t[:, i:i + 1], in_=yt[:],
                                            op=Alu.max, axis=mybir.AxisListType.X,
                                            apply_absolute_value=True)
                    # per-token scalars on [P, 1]
                    mse = small.tile([P, 1], fp32, tag="mse")
                    nc.vector.tensor_scalar(out=mse[:], in0=sumsq_t[:, i:i + 1],
                                            scalar1=1.0 / din, scalar2=EPS_NORM,
                                            op0=Alu.mult, op1=Alu.add)
                    sq = small.tile([P, 1], fp32, tag="sq")
                    nc.scalar.activation(sq[:], mse[:], Act.Sqrt)
                    d1 = small.tile([P, 1], fp32, tag="d1")
                    nc.vector.tensor_scalar(out=d1[:], in0=amax_t[:, i:i + 1],
                                            scalar1=1.0 / QB, scalar2=None,
                                            op0=Alu.mult)
                    # f1 = d1 + EPS_SCALE*sq ; m = 1/f1
                    e1 = small.tile([P, 1], fp32, tag="e1")
                    nc.vector.tensor_scalar(out=e1[:], in0=sq[:], scalar1=EPS_SCALE,
                                            scalar2=None, op0=Alu.mult)
                    f1 = small.tile([P, 1], fp32, tag="f1")
                    nc.vector.tensor_tensor(out=f1[:], in0=d1[:], in1=e1[:],
                                            op=Alu.add)
                    nc.vector.reciprocal(m_t[:, i:i + 1], f1[:])
                    rsq = small.tile([P, 1], fp32, tag="rsq")
                    nc.vector.reciprocal(rsq[:], sq[:])
                    xs0 = small.tile([P, 1], fp32, tag="xs0")
                    nc.vector.tensor_tensor(out=xs0[:], in0=d1[:], in1=rsq[:],
                                            op=Alu.mult)
                    # alpha = (xs0 + eps) * w_scale
                    nc.vector.tensor_scalar(out=alpha_t[:, i:i + 1], in0=xs0[:],
                                            scalar1=EPS_SCALE, scalar2=ws[:],
                                            op0=Alu.add, op1=Alu.mult)
                    # quantize
                    t1 = t1_pool.tile([P, din], fp32, tag="t1")
                    nc.vector.tensor_scalar(out=t1[:], in0=yt[:],
                                            scalar1=m_t[:, i:i + 1],
                                            scalar2=C_MAGIC,
                                            op0=Alu.mult, op1=Alu.add)
                    qt8 = q_pool.tile([P, din], bf16, tag="q")
                    nc.vector.tensor_scalar(out=qt8[:], in0=t1[:], scalar1=C_MAGIC,
                                            scalar2=None, op0=Alu.subtract)
                    # one xbar transpose for the whole tile: out[d_lo, k, t] =
                    # qt8[t, 128k + d_lo]  (verified blocked layout on HW)
                    qT = qt_pool.tile([P, k_tiles, P], bf16, tag="qT")
                    nc.sync.dma_start(out=qT[:], in_=qt8[:], transpose=True)
                    osb = out_pool.tile([P, o_sh], fp32, tag="o")
                    for oc in range(oc_chunks):
                        pt = psum_pool.tile([P, oc_sz], fp32, tag="ps")
                        for k in range(k_tiles):
                            nc.tensor.matmul(pt[:], qT[:, k, :],
                                             wq[k][:, oc * oc_sz:(oc + 1) * oc_sz],
                                             start=(k == 0), stop=(k == k_tiles - 1))
                        nc.scalar.activation(osb[:, oc * oc_sz:(oc + 1) * oc_sz],
                                             pt[:], Act.Copy,
                                             scale=alpha_t[:, i:i + 1])
                    nc.scalar.dma_start(out_d[i * P:(i + 1) * P, :], osb[:])

    nc.compile()
    return nc


# ---------------------------------------------------------------- host wrapper

_CACHED = {}


def _get_nc():
    if "nc" not in _CACHED:
        _CACHED["nc"] = build_bass()
    return _CACHED["nc"]


def kernel(x: np.ndarray, weight: np.ndarray, norm_weight: np.ndarray) -> np.ndarray:
    from concourse.bass_utils import run_bass_kernel_spmd

    assert x.shape == (B, S, DIN) and weight.shape == (DOUT, DIN)
    x_flat = np.ascontiguousarray(x.reshape(TOKENS, DIN), dtype=np.float32)
    w = np.ascontiguousarray(weight, dtype=np.float32)
    wt_full = np.ascontiguousarray(w.T)  # [DIN, DOUT]
    gw = np.ascontiguousarray(
        np.broadcast_to(norm_weight.astype(np.float32), (P, DIN)))

    in_maps = []
    for c in range(N_CORES):
        tg, og = divmod(c, OG)
        in_maps.append({
            "x": np.ascontiguousarray(x_flat[tg * T_SH:(tg + 1) * T_SH]),
            "wt": np.ascontiguousarray(wt_full[:, og * O_SH:(og + 1) * O_SH]),
            "wsc": np.ascontiguousarray(w[c * WSC_ROWS:(c + 1) * WSC_ROWS]),
            "gw": gw,
        })

    nc = _get_nc()
    res = run_bass_kernel_spmd(nc, in_maps, core_ids=list(range(N_CORES)))
    _CACHED["last_results"] = res

    out = np.empty((TOKENS, DOUT), dtype=np.float32)
    for c in range(N_CORES):
        tg, og = divmod(c, OG)
        out[tg * T_SH:(tg + 1) * T_SH, og * O_SH:(og + 1) * O_SH] = \
            res.results[c]["out"]
    return out.reshape(B, S, DOUT)
